# revision 18
# baseline (speedup 1.0000x reference)
"""Trainium2 Bass kernel for nn_CrossAttention_47502338294587.

Math: the reference cross-attention has a single KV position broadcast over
all T query positions.  Softmax over a row of identical logits is uniform,
so attention output == v for every query, and the whole module collapses to

    out[b, t, :] = (visual_features[b] @ Wv + bv) @ Wp + bp      (for all t)

independent of x / Wq / Wk.  The device computes the two projections and
broadcasts the per-batch row over the T axis; the host only does input
layout prep (incl. bf16 quantization of the weights) and shard re-assembly.

Sharding: tensor-parallel over the output channel dim C — core i computes
and writes out[:, :, i*128:(i+1)*128].

Per-core structure (matmuls bf16, PSUM accumulation fp32):
  warm:  idempotent rank-1 bias matmuls (psum = 1^T bv) + the vf^T
         transposes double as PE p-state warm-up while Wv streams in
  mm1:   vv = [1|vf] @ [bv; Wv]   moving Wv k-chunk-pairs (4KB DMA lines,
         HWDGE queues only -- SWDGE/gpsimd is ~2x slower per byte)
  tr:    vv^T chunks via PE transpose into ONE psum tile, one copy out
  mm2:   row = [1|vv] @ [bp; Wp[:,ci]]  (bias as rank-1 term)
  bcast: rhs4 = rep4(row)*sel (DVE), bc = ones^T @ rhs4 (one matmul),
         then 2 replicated out-DMAs (step-0 over the 8 t-chunks) on the
         two HWDGE queues
"""

import os
import sys

import numpy as np
import ml_dtypes

for _p in ("/opt/trn_rl_repo",):
    if _p not in sys.path and os.path.isdir(_p):
        sys.path.insert(0, _p)

B, T, C = 4, 1024, 1024
N_CORES = 8
CSH = C // N_CORES  # 128, C-shard per core
KC = C // 128  # 8 contraction chunks

BF16 = ml_dtypes.bfloat16

_BUILT = None


def build_nc():
    """Build + compile the Bass program (one NeuronCore's SPMD body)."""
    import concourse.bass as bass
    import concourse.mybir as mybir
    import concourse.tile as tile
    from concourse import bacc
    from concourse.bass import ts

    f32 = mybir.dt.float32
    bf16 = mybir.dt.bfloat16
    nc = bacc.Bacc("TRN2", target_bir_lowering=False, debug=False)

    # host pre-packs into the exact SBUF layouts (layout + bf16 quantization)
    wv_b = nc.dram_tensor("wv_b", [128, KC * C], bf16, kind="ExternalInput")
    wp_b = nc.dram_tensor("wp_b", [128, KC * CSH], bf16, kind="ExternalInput")
    vf_b = nc.dram_tensor("vf_b", [B, C], bf16, kind="ExternalInput")
    bvbp_b = nc.dram_tensor("bvbp_b", [1, C + CSH], bf16, kind="ExternalInput")
    eye_b = nc.dram_tensor("eye_b", [B, B], bf16, kind="ExternalInput")
    sel_b = nc.dram_tensor("sel_b", [B, B * CSH], bf16, kind="ExternalInput")
    # out[t, b, c_local]; host re-assembles full[b, t, ci] = out[t, b, :]
    out = nc.dram_tensor("out", [T, B, CSH], f32, kind="ExternalOutput")

    with tile.TileContext(nc) as tc:
        with tc.tile_pool(name="sb", bufs=1) as sb:
            # ---- SBUF tiles -------------------------------------------------
            wv_t = [
                sb.tile([128, C], bf16, name=f"wv{k}", tag=f"wv{k}")
                for k in range(KC)
            ]
            warm_t = sb.tile([1, 512], bf16, tag="warm")
            wp_t = sb.tile([128, KC, CSH], bf16, tag="wp_t")
            vf_t = sb.tile([B, C], bf16, tag="vf")
            vft_t = sb.tile([128, KC * B], bf16, tag="vft")
            bvbp_t = sb.tile([1, C + CSH], bf16, tag="bvbp")
            eye_t = sb.tile([B, B], bf16, tag="eye")
            sel_t = sb.tile([B, B * CSH], bf16, tag="sel")
            ones_t = sb.tile([1, B], bf16, tag="ones")
            ones_bc = sb.tile([B, 128], bf16, tag="ones_bc")
            vv_sb = sb.tile([B, C], bf16, tag="vv_sb")
            vvt_t = sb.tile([128, KC * B], bf16, tag="vvt")
            rhs4_t = sb.tile([B, B * CSH], bf16, tag="rhs4")
            bc_sb = sb.tile([128, B * CSH], f32, tag="bc")

            nc.gpsimd.memset(ones_t[:], 1.0)
            nc.gpsimd.memset(ones_bc[:], 1.0)
            nc.gpsimd.memset(warm_t[:], 0.5)

            # ---- DMA in: HWDGE queues (sync/scalar) carry the big loads -----
            # wv chunks first (mm1's critical path), singles for early arrival
            nc.sync.dma_start(wv_t[0][:], wv_b[:, 0:C])
            nc.sync.dma_start(wv_t[2][:], wv_b[:, 2 * C : 3 * C])
            nc.sync.dma_start(wv_t[4][:], wv_b[:, 4 * C : 5 * C])
            nc.sync.dma_start(wv_t[6][:], wv_b[:, 6 * C : 7 * C])
            nc.scalar.dma_start(wv_t[1][:], wv_b[:, C : 2 * C])
            nc.scalar.dma_start(wv_t[3][:], wv_b[:, 3 * C : 4 * C])
            nc.scalar.dma_start(wv_t[5][:], wv_b[:, 5 * C : 6 * C])
            # gpsimd/SWDGE: the small tensors + late-needed wv7/wp
            nc.gpsimd.dma_start(eye_t[:], eye_b[:, :])
            nc.gpsimd.dma_start(vf_t[:], vf_b[:, :])
            nc.gpsimd.dma_start(bvbp_t[:], bvbp_b[:, :])
            nc.gpsimd.dma_start(sel_t[:], sel_b[:, :])
            nc.gpsimd.dma_start(wv_t[7][:], wv_b[:, 7 * C : 8 * C])
            nc.gpsimd.dma_start(wp_t[:], wp_b.rearrange("p (k c) -> p k c", c=CSH))

            with (
                tc.tile_pool(name="pv", bufs=1, space="PSUM") as pv,
                tc.tile_pool(name="pf", bufs=1, space="PSUM") as pf,
                tc.tile_pool(name="pw", bufs=1, space="PSUM") as pw,
            ):
                psum_vv = [
                    pv.tile([B, 512], mybir.dt.float32, name=f"pvv{h}", tag=f"pvv{h}")
                    for h in range(2)
                ]
                # self-sufficient warm-up matmuls (memset inputs only -- no
                # DMA waits can be hoisted onto them): ramp the PE clock
                psum_warm = pw.tile([B, 512], mybir.dt.float32, tag="pwm")
                for _ in range(5):
                    nc.tensor.matmul(
                        psum_warm[:],
                        ones_t[0:1, :],
                        warm_t[0:1, :],
                        start=True,
                        stop=True,
                        skip_group_check=True,
                    )

                # ---- vf^T chunks via PE transpose (on the warming PE) -------
                psum_vft = pf.tile([128, KC * B], bf16, tag="pvf")
                for k in range(KC):
                    nc.tensor.transpose(
                        psum_vft[:, ts(k, B)], vf_t[0:B, ts(k, 128)], eye_t[0:B, 0:B]
                    )
                nc.vector.tensor_copy(vft_t[:], psum_vft[:])

                # rank-1 bias terms: psum_vv[h] = 1^T bv_half
                for h in range(2):
                    nc.tensor.matmul(
                        psum_vv[h][:],
                        ones_t[0:1, :],
                        bvbp_t[0:1, ts(h, 512)],
                        start=True,
                        stop=False,
                        skip_group_check=True,
                    )

                # ---- mm1: vv[b, n] = bv[n] + sum_k vf[b, k] Wv[k, n] --------
                for k in range(KC):
                    for h in range(2):
                        nc.tensor.matmul(
                            psum_vv[h][:],
                            vft_t[:, ts(k, B)],
                            wv_t[k][:, ts(h, 512)],
                            start=False,
                            stop=(k == KC - 1),
                            skip_group_check=True,
                        )
                # psum -> SBUF bf16 cast, split across DVE + Act engines
                nc.vector.tensor_copy(vv_sb[0:B, 0:512], psum_vv[0][:])
                nc.scalar.activation(
                    vv_sb[0:B, 512:1024],
                    psum_vv[1][:],
                    mybir.ActivationFunctionType.Copy,
                )

            # ---- transpose vv -> vv^T into one psum tile, one copy out ------
            with tc.tile_pool(name="pt", bufs=1, space="PSUM") as pt:
                psum_vvt = pt.tile([128, KC * B], bf16, tag="pvt")
                for k in range(KC):
                    nc.tensor.transpose(
                        psum_vvt[:, ts(k, B)], vv_sb[0:B, ts(k, 128)], eye_t[0:B, 0:B]
                    )
                nc.vector.tensor_copy(vvt_t[:], psum_vvt[:])

            # ---- mm2: row = bp + vv @ Wp[:,ci] ------------------------------
            with (
                tc.tile_pool(name="pr", bufs=1, space="PSUM") as pr,
                tc.tile_pool(name="pb", bufs=1, space="PSUM") as pb,
            ):
                psum_row = pr.tile([B, CSH], mybir.dt.float32, tag="pr")
                nc.tensor.matmul(
                    psum_row[:],
                    ones_t[0:1, :],
                    bvbp_t[0:1, C : C + CSH],
                    start=True,
                    stop=False,
                )
                for k in range(KC):
                    nc.tensor.matmul(
                        psum_row[:],
                        vvt_t[:, ts(k, B)],
                        wp_t[:, k, :],
                        start=False,
                        stop=(k == KC - 1),
                    )

                # ---- T-broadcast: bc[t, (q,c)] = row[q, c] ------------------
                pra = psum_row[:]
                prep = bass.AP(
                    pra.tensor, pra.offset, [list(pra.ap[0]), [0, B], list(pra.ap[1])]
                )
                nc.vector.tensor_mul(
                    rhs4_t[:].rearrange("p (q f) -> p q f", q=B),
                    prep,
                    sel_t[:].rearrange("p (q f) -> p q f", q=B),
                )
                psum_bc = pb.tile([128, B * CSH], mybir.dt.float32, tag="pb")
                nc.tensor.matmul(
                    psum_bc[:], ones_bc[0:B, :], rhs4_t[0:B, :], start=True, stop=True
                )
                # psum -> SBUF fp32 halves (DVE + Act), then replicated
                # out-DMAs (step-0 over the 8 t-chunks) on the HWDGE queues
                half = B * CSH // 2
                out_v = out.rearrange("(q p) b c -> p q (b c)", p=128)
                for i, deng in ((0, nc.sync), (1, nc.scalar)):
                    if i == 0:
                        nc.vector.tensor_copy(
                            bc_sb[:, i * half : (i + 1) * half],
                            psum_bc[:, i * half : (i + 1) * half],
                        )
                    else:
                        nc.scalar.activation(
                            bc_sb[:, i * half : (i + 1) * half],
                            psum_bc[:, i * half : (i + 1) * half],
                            mybir.ActivationFunctionType.Copy,
                        )
                    ap = bc_sb[:, i * half : (i + 1) * half]
                    rep = bass.AP(
                        ap.tensor, ap.offset, [list(ap.ap[0]), [0, KC], list(ap.ap[1])]
                    )
                    deng.dma_start(out_v[:, :, i * half : (i + 1) * half], rep)

    nc.compile()
    return nc


def _get_built():
    global _BUILT
    if _BUILT is None:
        _BUILT = build_nc()
    return _BUILT


def make_in_maps(inputs):
    vf = np.asarray(inputs["visual_features"], np.float32)
    wv = np.asarray(inputs["Wv"], np.float32)
    wp = np.asarray(inputs["Wp"], np.float32)
    bv = np.asarray(inputs["bv"], np.float32)
    bp = np.asarray(inputs["bp"], np.float32)
    # wv_b[p, k*C + n] = Wv[k*128 + p, n]
    wv_b = np.ascontiguousarray(
        wv.reshape(KC, 128, C).transpose(1, 0, 2).reshape(128, KC * C)
    ).astype(BF16)
    vf_b = vf.astype(BF16)
    eye_b = np.eye(B, dtype=np.float32).astype(BF16)
    sel_b = np.zeros((B, B * CSH), np.float32)
    for b in range(B):
        sel_b[b, b * CSH : (b + 1) * CSH] = 1.0
    sel_b = sel_b.astype(BF16)
    maps = []
    for i in range(N_CORES):
        ci = slice(i * CSH, (i + 1) * CSH)
        # wp_b[p, k*CSH + c] = Wp[k*128 + p, ci_c]
        wp_b = np.ascontiguousarray(
            wp[:, ci].reshape(KC, 128, CSH).transpose(1, 0, 2).reshape(128, KC * CSH)
        ).astype(BF16)
        bvbp_b = np.concatenate([bv, bp[ci]]).reshape(1, C + CSH).astype(BF16)
        maps.append(
            {
                "wv_b": wv_b,
                "wp_b": wp_b,
                "vf_b": vf_b,
                "bvbp_b": bvbp_b,
                "eye_b": eye_b,
                "sel_b": sel_b,
            }
        )
    return maps


def run(inputs, trace=False, **kw):
    from concourse.bass_utils import run_bass_kernel_spmd

    nc = _get_built()
    res = run_bass_kernel_spmd(
        nc,
        make_in_maps(inputs),
        core_ids=list(range(N_CORES)),
        trace=trace,
        **kw,
    )
    full = np.empty((B, T, C), np.float32)
    for i, r in enumerate(res.results):
        full[:, :, i * CSH : (i + 1) * CSH] = r["out"].transpose(1, 0, 2)
    return full, res


def kernel(**inputs) -> np.ndarray:
    full, _ = run(inputs, trace=False)
    return full


# revision 27
# speedup vs baseline: 1.0346x; 1.0346x over previous
"""Trainium2 Bass kernel for nn_CrossAttention_47502338294587.

Math: the reference cross-attention has a single KV position broadcast over
all T query positions.  Softmax over a row of identical logits is uniform,
so attention output == v for every query, and the whole module collapses to

    out[b, t, :] = (visual_features[b] @ Wv + bv) @ Wp + bp      (for all t)

independent of x / Wq / Wk.  The device computes the two projections and
broadcasts the per-batch row over the T axis; the host only does input
layout prep (incl. bf16 quantization of the weights) and shard re-assembly.

Sharding: tensor-parallel over the output channel dim C — core i computes
and writes out[:, :, i*128:(i+1)*128].

Per-core structure (matmuls bf16, PSUM accumulation fp32):
  warm:  idempotent rank-1 bias matmuls (psum = 1^T bv) + the vf^T
         transposes double as PE p-state warm-up while Wv streams in
  mm1:   vv = [1|vf] @ [bv; Wv]   moving Wv k-chunk-pairs (4KB DMA lines,
         HWDGE queues only -- SWDGE/gpsimd is ~2x slower per byte)
  tr:    vv^T chunks via PE transpose into ONE psum tile, one copy out
  mm2:   row = [1|vv] @ [bp; Wp[:,ci]]  (bias as rank-1 term)
  bcast: rhs4 = rep4(row)*sel (DVE), bc = ones^T @ rhs4 (one matmul),
         then 2 replicated out-DMAs (step-0 over the 8 t-chunks) on the
         two HWDGE queues
"""

import os
import sys

import numpy as np
import ml_dtypes

for _p in ("/opt/trn_rl_repo",):
    if _p not in sys.path and os.path.isdir(_p):
        sys.path.insert(0, _p)

B, T, C = 4, 1024, 1024
N_CORES = 8
CSH = C // N_CORES  # 128, C-shard per core
KC = C // 128  # 8 contraction chunks

BF16 = ml_dtypes.bfloat16

_BUILT = None


def build_nc():
    """Build + compile the Bass program (one NeuronCore's SPMD body)."""
    import concourse.bass as bass
    import concourse.mybir as mybir
    import concourse.tile as tile
    from concourse import bacc
    from concourse.bass import ts

    f32 = mybir.dt.float32
    bf16 = mybir.dt.bfloat16
    nc = bacc.Bacc("TRN2", target_bir_lowering=False, debug=False)

    # host pre-packs into the exact SBUF layouts (layout + bf16 quantization)
    wv_b = nc.dram_tensor("wv_b", [128, KC * C], bf16, kind="ExternalInput")
    wp_b = nc.dram_tensor("wp_b", [128, KC * CSH], bf16, kind="ExternalInput")
    # vf row b || eye row b: one DMA, 2056-byte lines
    vfeye_b = nc.dram_tensor("vfeye_b", [B, C + B], bf16, kind="ExternalInput")
    bvbp_b = nc.dram_tensor("bvbp_b", [1, C + CSH], bf16, kind="ExternalInput")
    sel_b = nc.dram_tensor("sel_b", [B, B * CSH], bf16, kind="ExternalInput")
    # out[t, b, c_local]; host re-assembles full[b, t, ci] = out[t, b, :]
    out = nc.dram_tensor("out", [T, B, CSH], f32, kind="ExternalOutput")

    with tile.TileContext(nc) as tc:
        with tc.tile_pool(name="sb", bufs=1) as sb:
            # ---- SBUF tiles -------------------------------------------------
            wv_t = [
                sb.tile([128, C], bf16, name=f"wv{k}", tag=f"wv{k}")
                for k in range(KC)
            ]
            warm_t = sb.tile([1, 512], bf16, tag="warm")
            wp_t = sb.tile([128, KC, CSH], bf16, tag="wp_t")
            vfeye_t = sb.tile([B, C + B], bf16, tag="vfeye")
            vft_t = sb.tile([128, KC * B], bf16, tag="vft")
            bvbp_t = sb.tile([1, C + CSH], bf16, tag="bvbp")
            sel_t = sb.tile([B, B * CSH], bf16, tag="sel")
            ones_t = sb.tile([1, B], bf16, tag="ones")
            ones_bc = sb.tile([B, 128], bf16, tag="ones_bc")
            vv_sb = sb.tile([B, C], bf16, tag="vv_sb")
            vvt_t = sb.tile([128, KC * B], bf16, tag="vvt")
            rhs4_t = sb.tile([B, B * CSH], bf16, tag="rhs4")
            bc_sb = sb.tile([128, B * CSH], f32, tag="bc")

            nc.gpsimd.memset(ones_t[:], 1.0)
            nc.gpsimd.memset(ones_bc[:], 1.0)
            nc.gpsimd.memset(warm_t[:], 0.5)

            # ---- DMA in: HWDGE queues (sync/scalar) carry everything early --
            # critical smalls first (tiny transfers), then wv chunk singles
            nc.sync.dma_start(vfeye_t[:], vfeye_b[:, :])
            nc.sync.dma_start(wv_t[0][:], wv_b[:, 0:C])
            nc.sync.dma_start(wv_t[2][:], wv_b[:, 2 * C : 3 * C])
            nc.sync.dma_start(wv_t[4][:], wv_b[:, 4 * C : 5 * C])
            nc.sync.dma_start(wv_t[6][:], wv_b[:, 6 * C : 7 * C])
            nc.scalar.dma_start(bvbp_t[:], bvbp_b[:, :])
            nc.scalar.dma_start(wv_t[1][:], wv_b[:, C : 2 * C])
            nc.scalar.dma_start(wv_t[3][:], wv_b[:, 3 * C : 4 * C])
            nc.scalar.dma_start(wv_t[5][:], wv_b[:, 5 * C : 6 * C])
            nc.scalar.dma_start(wv_t[7][:], wv_b[:, 7 * C : 8 * C])
            # gpsimd/SWDGE (slow): only the late-needed tensors
            nc.gpsimd.dma_start(sel_t[:], sel_b[:, :])
            nc.gpsimd.dma_start(wp_t[:], wp_b.rearrange("p (k c) -> p k c", c=CSH))

            with (
                tc.tile_pool(name="pv", bufs=1, space="PSUM") as pv,
                tc.tile_pool(name="pf", bufs=1, space="PSUM") as pf,
                tc.tile_pool(name="pw", bufs=1, space="PSUM") as pw,
            ):
                psum_vv = [
                    pv.tile([B, 512], mybir.dt.float32, name=f"pvv{h}", tag=f"pvv{h}")
                    for h in range(2)
                ]
                # self-sufficient warm-up matmuls (memset inputs only -- no
                # DMA waits can be hoisted onto them): ramp the PE clock
                psum_warm = pw.tile([B, 512], mybir.dt.float32, tag="pwm")
                for _ in range(5):
                    nc.tensor.matmul(
                        psum_warm[:],
                        ones_t[0:1, :],
                        warm_t[0:1, :],
                        start=True,
                        stop=True,
                        skip_group_check=True,
                    )

                # ---- vf^T chunks via PE transpose (on the warming PE) -------
                psum_vft = pf.tile([128, KC * B], bf16, tag="pvf")
                for k in range(KC):
                    nc.tensor.transpose(
                        psum_vft[:, ts(k, B)],
                        vfeye_t[0:B, ts(k, 128)],
                        vfeye_t[0:B, C : C + B],
                    )
                nc.vector.tensor_copy(vft_t[:], psum_vft[:])

                # rank-1 bias terms: psum_vv[h] = 1^T bv_half
                for h in range(2):
                    nc.tensor.matmul(
                        psum_vv[h][:],
                        ones_t[0:1, :],
                        bvbp_t[0:1, ts(h, 512)],
                        start=True,
                        stop=False,
                        skip_group_check=True,
                    )

                # ---- mm1: vv[b, n] = bv[n] + sum_k vf[b, k] Wv[k, n] --------
                for k in range(KC):
                    for h in range(2):
                        nc.tensor.matmul(
                            psum_vv[h][:],
                            vft_t[:, ts(k, B)],
                            wv_t[k][:, ts(h, 512)],
                            start=False,
                            stop=(k == KC - 1),
                            skip_group_check=True,
                        )
                # psum -> SBUF bf16 cast, split across DVE + Act engines
                nc.vector.tensor_copy(vv_sb[0:B, 0:512], psum_vv[0][:])
                nc.scalar.activation(
                    vv_sb[0:B, 512:1024],
                    psum_vv[1][:],
                    mybir.ActivationFunctionType.Copy,
                )

            # ---- transpose vv -> vv^T into one psum tile, one copy out ------
            with tc.tile_pool(name="pt", bufs=1, space="PSUM") as pt:
                psum_vvt = pt.tile([128, KC * B], bf16, tag="pvt")
                for k in range(KC):
                    nc.tensor.transpose(
                        psum_vvt[:, ts(k, B)],
                        vv_sb[0:B, ts(k, 128)],
                        vfeye_t[0:B, C : C + B],
                    )
                nc.vector.tensor_copy(vvt_t[:], psum_vvt[:])

            # ---- mm2: row = bp + vv @ Wp[:,ci] ------------------------------
            with (
                tc.tile_pool(name="pr", bufs=1, space="PSUM") as pr,
                tc.tile_pool(name="pb", bufs=1, space="PSUM") as pb,
            ):
                psum_row = pr.tile([B, CSH], mybir.dt.float32, tag="pr")
                nc.tensor.matmul(
                    psum_row[:],
                    ones_t[0:1, :],
                    bvbp_t[0:1, C : C + CSH],
                    start=True,
                    stop=False,
                )
                for k in range(KC):
                    nc.tensor.matmul(
                        psum_row[:],
                        vvt_t[:, ts(k, B)],
                        wp_t[:, k, :],
                        start=False,
                        stop=(k == KC - 1),
                    )

                # ---- T-broadcast: bc[t, (q,c)] = row[q, c] ------------------
                pra = psum_row[:]
                prep = bass.AP(
                    pra.tensor, pra.offset, [list(pra.ap[0]), [0, B], list(pra.ap[1])]
                )
                nc.vector.tensor_mul(
                    rhs4_t[:].rearrange("p (q f) -> p q f", q=B),
                    prep,
                    sel_t[:].rearrange("p (q f) -> p q f", q=B),
                )
                psum_bc = pb.tile([128, B * CSH], mybir.dt.float32, tag="pb")
                nc.tensor.matmul(
                    psum_bc[:], ones_bc[0:B, :], rhs4_t[0:B, :], start=True, stop=True
                )
                # psum -> SBUF fp32 quarters (DVE + Act alternating), each
                # followed by its replicated out-DMA (step-0 over the 8
                # t-chunks) on the two HWDGE queues
                qr = B * CSH // 4
                out_v = out.rearrange("(q p) b c -> p q (b c)", p=128)
                for i, (ceng, deng) in enumerate(
                    (
                        (nc.vector, nc.sync),
                        (nc.scalar, nc.scalar),
                        (nc.vector, nc.sync),
                        (nc.scalar, nc.scalar),
                    )
                ):
                    if ceng is nc.vector:
                        ceng.tensor_copy(
                            bc_sb[:, i * qr : (i + 1) * qr],
                            psum_bc[:, i * qr : (i + 1) * qr],
                        )
                    else:
                        ceng.activation(
                            bc_sb[:, i * qr : (i + 1) * qr],
                            psum_bc[:, i * qr : (i + 1) * qr],
                            mybir.ActivationFunctionType.Copy,
                        )
                    ap = bc_sb[:, i * qr : (i + 1) * qr]
                    rep = bass.AP(
                        ap.tensor, ap.offset, [list(ap.ap[0]), [0, KC], list(ap.ap[1])]
                    )
                    deng.dma_start(out_v[:, :, i * qr : (i + 1) * qr], rep)

    nc.compile()
    return nc


def _get_built():
    global _BUILT
    if _BUILT is None:
        _BUILT = build_nc()
    return _BUILT


def make_in_maps(inputs):
    vf = np.asarray(inputs["visual_features"], np.float32)
    wv = np.asarray(inputs["Wv"], np.float32)
    wp = np.asarray(inputs["Wp"], np.float32)
    bv = np.asarray(inputs["bv"], np.float32)
    bp = np.asarray(inputs["bp"], np.float32)
    # wv_b[p, k*C + n] = Wv[k*128 + p, n]
    wv_b = np.ascontiguousarray(
        wv.reshape(KC, 128, C).transpose(1, 0, 2).reshape(128, KC * C)
    ).astype(BF16)
    vfeye_b = np.concatenate([vf, np.eye(B, dtype=np.float32)], axis=1).astype(BF16)
    sel_b = np.zeros((B, B * CSH), np.float32)
    for b in range(B):
        sel_b[b, b * CSH : (b + 1) * CSH] = 1.0
    sel_b = sel_b.astype(BF16)
    maps = []
    for i in range(N_CORES):
        ci = slice(i * CSH, (i + 1) * CSH)
        # wp_b[p, k*CSH + c] = Wp[k*128 + p, ci_c]
        wp_b = np.ascontiguousarray(
            wp[:, ci].reshape(KC, 128, CSH).transpose(1, 0, 2).reshape(128, KC * CSH)
        ).astype(BF16)
        bvbp_b = np.concatenate([bv, bp[ci]]).reshape(1, C + CSH).astype(BF16)
        maps.append(
            {
                "wv_b": wv_b,
                "wp_b": wp_b,
                "vfeye_b": vfeye_b,
                "bvbp_b": bvbp_b,
                "sel_b": sel_b,
            }
        )
    return maps


def run(inputs, trace=False, **kw):
    from concourse.bass_utils import run_bass_kernel_spmd

    nc = _get_built()
    res = run_bass_kernel_spmd(
        nc,
        make_in_maps(inputs),
        core_ids=list(range(N_CORES)),
        trace=trace,
        **kw,
    )
    full = np.empty((B, T, C), np.float32)
    for i, r in enumerate(res.results):
        full[:, :, i * CSH : (i + 1) * CSH] = r["out"].transpose(1, 0, 2)
    return full, res


def kernel(**inputs) -> np.ndarray:
    full, _ = run(inputs, trace=False)
    return full


# revision 32
# speedup vs baseline: 1.1674x; 1.1283x over previous
"""Trainium2 Bass kernel for nn_CrossAttention_47502338294587.

Math: the reference cross-attention has a single KV position broadcast over
all T query positions.  Softmax over a row of identical logits is uniform,
so attention output == v for every query, and the whole module collapses to

    out[b, t, :] = (visual_features[b] @ Wv + bv) @ Wp + bp      (for all t)

independent of x / Wq / Wk.  The device computes the two projections and
broadcasts the per-batch row over the T axis; the host only does input
layout prep (incl. bf16 quantization of the weights) and shard re-assembly.

Sharding: tensor-parallel over the output channel dim C — core i computes
and writes out[:, :, i*128:(i+1)*128].

Per-core structure (matmuls bf16, PSUM accumulation fp32):
  warm:  idempotent rank-1 bias matmuls (psum = 1^T bv) + the vf^T
         transposes double as PE p-state warm-up while Wv streams in
  mm1:   vv = [1|vf] @ [bv; Wv]   moving Wv k-chunk-pairs (4KB DMA lines,
         HWDGE queues only -- SWDGE/gpsimd is ~2x slower per byte)
  tr:    vv^T chunks via PE transpose into ONE psum tile, one copy out
  mm2:   row = [1|vv] @ [bp; Wp[:,ci]]  (bias as rank-1 term)
  bcast: rhs4 = rep4(row)*sel (DVE), bc = ones^T @ rhs4 (one matmul),
         then 2 replicated out-DMAs (step-0 over the 8 t-chunks) on the
         two HWDGE queues
"""

import os
import sys

import numpy as np
import ml_dtypes

for _p in ("/opt/trn_rl_repo",):
    if _p not in sys.path and os.path.isdir(_p):
        sys.path.insert(0, _p)

B, T, C = 4, 1024, 1024
N_CORES = 8
CSH = C // N_CORES  # 128, C-shard per core
KC = C // 128  # 8 contraction chunks

BF16 = ml_dtypes.bfloat16

_BUILT = None


def build_nc():
    """Build + compile the Bass program (one NeuronCore's SPMD body)."""
    import concourse.bass as bass
    import concourse.mybir as mybir
    import concourse.tile as tile
    from concourse import bacc
    from concourse.bass import ts

    f32 = mybir.dt.float32
    bf16 = mybir.dt.bfloat16
    nc = bacc.Bacc("TRN2", target_bir_lowering=False, debug=False)

    # host pre-packs into the exact SBUF layouts (layout + bf16 quantization)
    wv_b = nc.dram_tensor("wv_b", [128, KC * C], bf16, kind="ExternalInput")
    wp_b = nc.dram_tensor("wp_b", [128, KC * CSH], bf16, kind="ExternalInput")
    # vf row b || eye row b: one DMA, 2056-byte lines
    vfeye_b = nc.dram_tensor("vfeye_b", [B, C + B], bf16, kind="ExternalInput")
    bvbp_b = nc.dram_tensor("bvbp_b", [1, C + CSH], bf16, kind="ExternalInput")
    sel_b = nc.dram_tensor("sel_b", [B, B * CSH], bf16, kind="ExternalInput")
    # out[t, b, c_local] in bf16 (host widens to fp32 -- exact cast);
    # host re-assembles full[b, t, ci] = out[t, b, :]
    out = nc.dram_tensor("out", [T, B, CSH], bf16, kind="ExternalOutput")

    with tile.TileContext(nc) as tc:
        with tc.tile_pool(name="sb", bufs=1) as sb:
            # ---- SBUF tiles -------------------------------------------------
            wv_t = [
                sb.tile([128, C], bf16, name=f"wv{k}", tag=f"wv{k}")
                for k in range(KC)
            ]
            warm_t = sb.tile([1, 512], bf16, tag="warm")
            wp_t = sb.tile([128, KC, CSH], bf16, tag="wp_t")
            vfeye_t = sb.tile([B, C + B], bf16, tag="vfeye")
            vft_t = sb.tile([128, KC * B], bf16, tag="vft")
            bvbp_t = sb.tile([1, C + CSH], bf16, tag="bvbp")
            sel_t = sb.tile([B, B * CSH], bf16, tag="sel")
            ones_t = sb.tile([1, B], bf16, tag="ones")
            ones_bc = sb.tile([B, 128], bf16, tag="ones_bc")
            vv_sb = sb.tile([B, C], bf16, tag="vv_sb")
            vvt_t = sb.tile([128, KC * B], bf16, tag="vvt")
            rhs4_t = sb.tile([B, B * CSH], bf16, tag="rhs4")
            bc_sb = sb.tile([128, B * CSH], bf16, tag="bc")

            nc.gpsimd.memset(ones_t[:], 1.0)
            nc.gpsimd.memset(ones_bc[:], 1.0)
            nc.gpsimd.memset(warm_t[:], 0.5)

            # ---- DMA in: ONE HWDGE queue streams everything critical --------
            # (a single queue sustains ~320+ GB/s; two queues thrash to ~240)
            nc.sync.dma_start(vfeye_t[:], vfeye_b[:, :])
            for k in range(KC):
                nc.sync.dma_start(wv_t[k][:], wv_b[:, k * C : (k + 1) * C])
            nc.scalar.dma_start(bvbp_t[:], bvbp_b[:, :])
            # gpsimd/SWDGE (slow): only the late-needed tensors
            nc.gpsimd.dma_start(sel_t[:], sel_b[:, :])
            nc.gpsimd.dma_start(wp_t[:], wp_b.rearrange("p (k c) -> p k c", c=CSH))

            with (
                tc.tile_pool(name="pv", bufs=1, space="PSUM") as pv,
                tc.tile_pool(name="pf", bufs=1, space="PSUM") as pf,
                tc.tile_pool(name="pw", bufs=1, space="PSUM") as pw,
            ):
                psum_vv = [
                    pv.tile([B, 512], mybir.dt.float32, name=f"pvv{h}", tag=f"pvv{h}")
                    for h in range(2)
                ]
                # self-sufficient warm-up matmuls (memset inputs only -- no
                # DMA waits can be hoisted onto them): ramp the PE clock
                psum_warm = pw.tile([B, 512], mybir.dt.float32, tag="pwm")
                for _ in range(5):
                    nc.tensor.matmul(
                        psum_warm[:],
                        ones_t[0:1, :],
                        warm_t[0:1, :],
                        start=True,
                        stop=True,
                        skip_group_check=True,
                    )

                # ---- vf^T chunks via PE transpose (on the warming PE) -------
                psum_vft = pf.tile([128, KC * B], bf16, tag="pvf")
                for k in range(KC):
                    nc.tensor.transpose(
                        psum_vft[:, ts(k, B)],
                        vfeye_t[0:B, ts(k, 128)],
                        vfeye_t[0:B, C : C + B],
                    )
                nc.vector.tensor_copy(vft_t[:], psum_vft[:])

                # rank-1 bias terms: psum_vv[h] = 1^T bv_half
                for h in range(2):
                    nc.tensor.matmul(
                        psum_vv[h][:],
                        ones_t[0:1, :],
                        bvbp_t[0:1, ts(h, 512)],
                        start=True,
                        stop=False,
                        skip_group_check=True,
                    )

                # ---- mm1: vv[b, n] = bv[n] + sum_k vf[b, k] Wv[k, n] --------
                for k in range(KC):
                    for h in range(2):
                        nc.tensor.matmul(
                            psum_vv[h][:],
                            vft_t[:, ts(k, B)],
                            wv_t[k][:, ts(h, 512)],
                            start=False,
                            stop=(k == KC - 1),
                            skip_group_check=True,
                        )
                # psum -> SBUF bf16 cast, split across DVE + Act engines
                nc.vector.tensor_copy(vv_sb[0:B, 0:512], psum_vv[0][:])
                nc.scalar.activation(
                    vv_sb[0:B, 512:1024],
                    psum_vv[1][:],
                    mybir.ActivationFunctionType.Copy,
                )

            # ---- transpose vv -> vv^T into one psum tile, one copy out ------
            with tc.tile_pool(name="pt", bufs=1, space="PSUM") as pt:
                psum_vvt = pt.tile([128, KC * B], bf16, tag="pvt")
                for k in range(KC):
                    nc.tensor.transpose(
                        psum_vvt[:, ts(k, B)],
                        vv_sb[0:B, ts(k, 128)],
                        vfeye_t[0:B, C : C + B],
                    )
                nc.vector.tensor_copy(vvt_t[:], psum_vvt[:])

            # ---- mm2: row = bp + vv @ Wp[:,ci] ------------------------------
            with (
                tc.tile_pool(name="pr", bufs=1, space="PSUM") as pr,
                tc.tile_pool(name="pb", bufs=1, space="PSUM") as pb,
            ):
                psum_row = pr.tile([B, CSH], mybir.dt.float32, tag="pr")
                nc.tensor.matmul(
                    psum_row[:],
                    ones_t[0:1, :],
                    bvbp_t[0:1, C : C + CSH],
                    start=True,
                    stop=False,
                )
                for k in range(KC):
                    nc.tensor.matmul(
                        psum_row[:],
                        vvt_t[:, ts(k, B)],
                        wp_t[:, k, :],
                        start=False,
                        stop=(k == KC - 1),
                    )

                # ---- T-broadcast: bc[t, (q,c)] = row[q, c] ------------------
                pra = psum_row[:]
                prep = bass.AP(
                    pra.tensor, pra.offset, [list(pra.ap[0]), [0, B], list(pra.ap[1])]
                )
                nc.vector.tensor_mul(
                    rhs4_t[:].rearrange("p (q f) -> p q f", q=B),
                    prep,
                    sel_t[:].rearrange("p (q f) -> p q f", q=B),
                )
                psum_bc = pb.tile([128, B * CSH], mybir.dt.float32, tag="pb")
                nc.tensor.matmul(
                    psum_bc[:], ones_bc[0:B, :], rhs4_t[0:B, :], start=True, stop=True
                )
                # psum -> SBUF bf16 halves (DVE + Act in parallel), then the
                # replicated out-DMAs (step-0 over the 8 t-chunks), both on
                # the sync queue (single-queue streaming)
                half = B * CSH // 2
                out_v = out.rearrange("(q p) b c -> p q (b c)", p=128)
                nc.vector.tensor_copy(bc_sb[:, 0:half], psum_bc[:, 0:half])
                nc.scalar.activation(
                    bc_sb[:, half:],
                    psum_bc[:, half:],
                    mybir.ActivationFunctionType.Copy,
                )
                for i in range(2):
                    ap = bc_sb[:, i * half : (i + 1) * half]
                    rep = bass.AP(
                        ap.tensor, ap.offset, [list(ap.ap[0]), [0, KC], list(ap.ap[1])]
                    )
                    nc.sync.dma_start(out_v[:, :, i * half : (i + 1) * half], rep)

    nc.compile()
    return nc


def _get_built():
    global _BUILT
    if _BUILT is None:
        _BUILT = build_nc()
    return _BUILT


def make_in_maps(inputs):
    vf = np.asarray(inputs["visual_features"], np.float32)
    wv = np.asarray(inputs["Wv"], np.float32)
    wp = np.asarray(inputs["Wp"], np.float32)
    bv = np.asarray(inputs["bv"], np.float32)
    bp = np.asarray(inputs["bp"], np.float32)
    # wv_b[p, k*C + n] = Wv[k*128 + p, n]
    wv_b = np.ascontiguousarray(
        wv.reshape(KC, 128, C).transpose(1, 0, 2).reshape(128, KC * C)
    ).astype(BF16)
    vfeye_b = np.concatenate([vf, np.eye(B, dtype=np.float32)], axis=1).astype(BF16)
    sel_b = np.zeros((B, B * CSH), np.float32)
    for b in range(B):
        sel_b[b, b * CSH : (b + 1) * CSH] = 1.0
    sel_b = sel_b.astype(BF16)
    maps = []
    for i in range(N_CORES):
        ci = slice(i * CSH, (i + 1) * CSH)
        # wp_b[p, k*CSH + c] = Wp[k*128 + p, ci_c]
        wp_b = np.ascontiguousarray(
            wp[:, ci].reshape(KC, 128, CSH).transpose(1, 0, 2).reshape(128, KC * CSH)
        ).astype(BF16)
        bvbp_b = np.concatenate([bv, bp[ci]]).reshape(1, C + CSH).astype(BF16)
        maps.append(
            {
                "wv_b": wv_b,
                "wp_b": wp_b,
                "vfeye_b": vfeye_b,
                "bvbp_b": bvbp_b,
                "sel_b": sel_b,
            }
        )
    return maps


def run(inputs, trace=False, **kw):
    from concourse.bass_utils import run_bass_kernel_spmd

    nc = _get_built()
    res = run_bass_kernel_spmd(
        nc,
        make_in_maps(inputs),
        core_ids=list(range(N_CORES)),
        trace=trace,
        **kw,
    )
    full = np.empty((B, T, C), np.float32)
    for i, r in enumerate(res.results):
        full[:, :, i * CSH : (i + 1) * CSH] = (
            r["out"].astype(np.float32).transpose(1, 0, 2)
        )
    return full, res


def kernel(**inputs) -> np.ndarray:
    full, _ = run(inputs, trace=False)
    return full
